# revision 1
# baseline (speedup 1.0000x reference)
"""Trainium2 Bass kernel for a 2-layer dense-graph GAT (nn_GAT_79224966742097).

Reference (per batch sample, n=2048 nodes):
  layer0: x[2048,64] -> instance_norm over nodes -> 4-head GAT (f_out=64)
          -> gelu(concat heads + bias0) -> [2048, 256]
  layer1: instance_norm -> 1-head GAT (f_in=256, f_out=64) -> + bias1

Sharding: data-parallel over batch (bs=8) across the 8 NeuronCores; weights
replicated.  All tensors stay in SBUF; the n x n attention is never in HBM.

Attention layout: logits are computed TRANSPOSED, z^T[m, n] (m = softmax
reduction index on partitions) via z^T = a_sum-slice (stationary) @ hpT
(moving), where a_sum = a_src + a_dst is folded on host (the reference's
s+d terms share hp).  E = exp(leaky_relu(z^T)) and then
    U[n, o], denom[n] = sum_m E[m, n] * [hp | 1][m, o]
on the PE (K=128 accumulation), so the softmax denominator falls out of the
matmul's ones column -- no cross-partition reduction, no transpose of E.
Softmax max-subtraction is skipped: logits are bounded (|z| < 16) for
instance-normalized inputs, so exp stays far from fp32 limits.

Perf notes:
 * fp32r matmuls: full PE rate at moving-dim 512 (fp32 is 4x slower).
 * 2-head row-group packing: heads of a pair live at partitions 0-63 /
   64-127 (K=64 each), their logit matmuls run concurrently in the PE.
 * leaky-relu: ACT Prelu (same ACT LUT set as Exp -> no table reloads) for
   a fraction of tiles, DVE copy+fused (z*0.2 max z) for the rest --
   balances the two elementwise engines.
 * gelu is deferred and batched (4 big in-place ops) to avoid Exp<->Gelu
   ACT-table thrash inside the attention stream.
"""

import numpy as np

import concourse.bass as bass
import concourse.bacc as bacc
import concourse.mybir as mybir
import concourse.tile as tile
from contextlib import ExitStack
from concourse.masks import make_identity

F32 = mybir.dt.float32
F32R = mybir.dt.float32r
F16 = mybir.dt.float16
AX = mybir.AluOpType

N = 2048          # nodes
F0 = 64           # layer0 f_in
H0 = 4            # layer0 heads
FO = 64           # f_out (both layers)
NT = N // 128     # 16 node tiles
EPS = 1e-5
NEG_SLOPE = 0.2
N_CORES = 8

# lrelu engine split: tile index t uses ACT-Prelu when
# (t % ACT_LRELU_DEN) < ACT_LRELU_NUM, else DVE copy + fused (z*0.2 max z).
ACT_LRELU_NUM = 1
ACT_LRELU_DEN = 1


def _mm(nc, out, lhsT, rhs, start=True, stop=True, tile_position=None):
    nc.tensor.matmul(out, lhsT.bitcast(F32R), rhs.bitcast(F32R),
                     start=start, stop=stop, tile_position=tile_position)


def build_bass(sim_safe=False, act_lrelu=None, repeat=1):
    """Emit the full SPMD program for one core. Returns compiled nc."""
    nc = bacc.Bacc("TRN2", debug=False)

    x_d = nc.dram_tensor("x", [N, F0], F32, kind="ExternalInput")
    a0_d = nc.dram_tensor("a0", [128, 2, N], F32, kind="ExternalInput")
    w0_d = nc.dram_tensor("w0", [64, 2, 128], F32, kind="ExternalInput")
    b0_d = nc.dram_tensor("b0", [64, 1], F32, kind="ExternalInput")
    a1_d = nc.dram_tensor("a1", [128, N], F32, kind="ExternalInput")
    w1_d = nc.dram_tensor("w1", [64, 4, 128], F32, kind="ExternalInput")
    out_d = nc.dram_tensor("out", [N, FO], F32, kind="ExternalOutput")

    gelu_func = (mybir.ActivationFunctionType.Identity if sim_safe
                 else mybir.ActivationFunctionType.Gelu)
    lrelu_func = mybir.ActivationFunctionType.Prelu
    if act_lrelu is None:
        act_lrelu = (ACT_LRELU_NUM, ACT_LRELU_DEN)

    with tile.TileContext(nc) as tc, ExitStack() as ctx:
        const = ctx.enter_context(tc.tile_pool(name="const", bufs=1))
        sb = ctx.enter_context(tc.tile_pool(name="sb", bufs=1))
        ps = ctx.enter_context(tc.tile_pool(name="ps", bufs=2, space="PSUM"))
        ep = ctx.enter_context(tc.tile_pool(name="ep", bufs=2))
        small = ctx.enter_context(tc.tile_pool(name="small", bufs=2))
        dram = ctx.enter_context(tc.tile_pool(name="dram", bufs=2, space="DRAM"))

        # ---------- constants & weights ----------
        for _rep in range(repeat):
            body(nc, tc, const, sb, ps, ep, small, dram, sim_safe,
                 act_lrelu, gelu_func, lrelu_func,
                 x_d, a0_d, w0_d, b0_d, a1_d, w1_d, out_d)
    nc.compile()
    return nc


def body(nc, tc, const, sb, ps, ep, small, dram, sim_safe, act_lrelu,
         gelu_func, lrelu_func, x_d, a0_d, w0_d, b0_d, a1_d, w1_d, out_d):
        ident = const.tile([128, 128], F32, name="ident", uniquify=True)
        make_identity(nc, ident)
        eps_sb = const.tile([128, 1], F32)
        nc.vector.memset(eps_sb, EPS)
        ones_sb = const.tile([128, 64], F32)
        nc.vector.memset(ones_sb, 1.0)

        b0_sb = const.tile([64, 1], F32)
        nc.sync.dma_start(out=b0_sb, in_=b0_d.ap())
        w0_sb = const.tile([64, 2, 128], F32)
        nc.sync.dma_start(out=w0_sb.bitcast(F32R), in_=w0_d.ap().bitcast(F32R))
        w1_sb = const.tile([64, 4, 128], F32)
        nc.sync.dma_start(out=w1_sb.bitcast(F32R), in_=w1_d.ap().bitcast(F32R))
        a0_sb = sb.tile([128, 2, N], F32)
        nc.sync.dma_start(out=a0_sb.bitcast(F32R), in_=a0_d.ap().bitcast(F32R))
        a1_sb = sb.tile([128, N], F32)
        nc.sync.dma_start(out=a1_sb.bitcast(F32R), in_=a1_d.ap().bitcast(F32R))

        # ---------- load x, transpose to [f, n], instance-norm ----------
        x_nt = ep.tile([128, NT, F0], F32, tag="e")
        nc.sync.dma_start(out=x_nt,
                          in_=x_d.ap().rearrange("(t p) f -> p t f", p=128))
        xT = sb.tile([64, N], F32)   # becomes x_normT in place
        xt_ps = ps.tile([64, N], F32, tag="z", bufs=1)
        for t in range(NT):
            nc.tensor.transpose(xt_ps[:, t * 128:(t + 1) * 128],
                                x_nt[:, t, :], ident)
        nc.vector.tensor_copy(out=xT.bitcast(F32R), in_=xt_ps)

        st0 = small.tile([64, 4, 6], F32, tag="st")
        for c in range(4):
            nc.vector.bn_stats(out=st0[:, c, :], in_=xT[:, c * 512:(c + 1) * 512])
        mv0 = small.tile([64, 2], F32, tag="mv")
        nc.vector.bn_aggr(out=mv0, in_=st0)
        sd0 = small.tile([64, 1], F32, tag="sd")
        nc.scalar.activation(out=sd0, in_=mv0[:, 1:2],
                             func=mybir.ActivationFunctionType.Sqrt,
                             bias=eps_sb[0:64, :])
        rs0 = small.tile([64, 1], F32, tag="rs")
        nc.vector.reciprocal(out=rs0, in_=sd0)
        nc.vector.tensor_scalar(out=xT.bitcast(F32R), in0=xT, scalar1=mv0[:, 0:1],
                                scalar2=rs0, op0=AX.subtract, op1=AX.mult)

        # ---------- layer0 h': hpT packed [128, j, n] + hp rows ----------
        # head h lives at partitions 64*(h%2)..+64, pair j = h//2
        hpT = sb.tile([128, 2, N], F32)
        for j in range(2):
            hp_ps = ps.tile([128, 2048], F32, tag="z", bufs=1, name="hp_ps")
            for c in range(4):
                _mm(nc, hp_ps[:, c * 512:(c + 1) * 512],
                    w0_sb[:, j, :], xT[:, c * 512:(c + 1) * 512])
            nc.scalar.copy(out=hpT[:, j, :].bitcast(F32R), in_=hp_ps)

        hp1 = sb.tile([128, H0, NT, 65], F32)
        nc.vector.tensor_copy(out=hp1[:, :, :, 64].bitcast(F32R),
                              in_=ones_sb.rearrange("p (a b) -> p a b", a=H0))
        w0cat = w0_sb.rearrange("p a b -> p (a b)")   # [64, 256] all heads
        for g in range(4):
            hpr_ps = ps.tile([128, 4, 256], F32, tag="z", bufs=1)
            for k in range(4):
                m = 4 * g + k
                _mm(nc, hpr_ps[:, k, :],
                    xT[:, m * 128:(m + 1) * 128], w0cat)
            nc.vector.tensor_copy(
                out=hp1[:, :, 4 * g:4 * g + 4, 0:64].bitcast(F32R),
                in_=hpr_ps.rearrange("p m (h o) -> p h m o", h=H0))

        # ---------- attention block (shared by both layers) ----------
        lrelu_ctr = [0]

        def lrelu_on_act():
            t = lrelu_ctr[0]
            lrelu_ctr[0] += 1
            if sim_safe:
                return False
            return (t % act_lrelu[1]) < act_lrelu[0]

        def attention(z_mms, u_mms, n_m, finish):
            """Generic fused z -> lrelu -> exp -> U loop (n-half granularity,
            two 512-wide quarters per z psum tile)."""
            zs = {}

            def emit_z(m):
                zt = ps.tile([128, 2048], F32, tag="z", bufs=1, name="zt")
                z_mms(m, zt)
                zs[m] = zt

            ets = {}
            EB = 4   # m-units per exp batch

            def process(m):
                if m % EB == 0:
                    ets[m // EB] = ep.tile([128, EB * 2048], F32, tag="e",
                                           name="et")
                et = ets[m // EB]
                base = (m % EB) * 2048
                zt = zs.pop(m)
                dst = et[:, base:base + 2048]
                if lrelu_on_act():
                    nc.scalar.activation(out=dst.bitcast(F32R), in_=zt,
                                         func=lrelu_func, alpha=NEG_SLOPE)
                else:
                    nc.vector.tensor_copy(out=dst.bitcast(F32R), in_=zt)
                    nc.vector.scalar_tensor_tensor(
                        out=dst.bitcast(F32R), in0=dst, scalar=NEG_SLOPE,
                        in1=dst, op0=AX.mult, op1=AX.max)
                if m % EB == EB - 1:
                    nc.scalar.activation(out=et.bitcast(F32R), in_=et,
                                         func=mybir.ActivationFunctionType.Exp)
                    for mu in range(m - EB + 1, m + 1):
                        b2 = (mu % EB) * 2048
                        for q in range(2):
                            u_mms(mu, q,
                                  et[:, b2 + q * 1024:b2 + (q + 1) * 1024],
                                  mu == 0, mu == n_m - 1)
                    ets.pop(m // EB)

            emit_z(0)
            for m in range(n_m):
                if m + 1 < n_m:
                    emit_z(m + 1)
                process(m)
            finish()

        def div_to(u_ps, dst_ap):
            """dst = U / denom (denominator broadcast via DRAM bounce)."""
            r1 = small.tile([1, 1024], F32, tag="r1")
            nc.vector.reciprocal(out=r1, in_=u_ps[64:65, :])
            r1_dram = dram.tile([1, 1024], F32, tag="r1d")
            nc.sync.dma_start(out=r1_dram, in_=r1)
            rb_sb = ep.tile([64, 1024], F32, tag="e")
            bc_ap = bass.AP(tensor=r1_dram.tensor, offset=r1_dram.offset,
                            ap=[[0, 64], [1, 1024]])
            nc.sync.dma_start(out=rb_sb, in_=bc_ap)
            nc.vector.tensor_mul(out=dst_ap.bitcast(F32R),
                                 in0=u_ps[0:64, :], in1=rb_sb)

        # ---------- layer0 attention -> x1T [64, h, n] (pre-gelu) ----------
        x1T = sb.tile([64, H0, N], F32)

        for j in range(2):
            for half in range(2):
                u_ps = ps.tile([65, 2048], F32, tag="u", bufs=1,
                               name=f"u{j}{half}")

                def z_mms(m, zt, j=j, half=half):
                    for q in range(2):
                        n_off = half * 1024 + q * 512
                        for s in range(2):
                            _mm(nc, zt[:, q * 1024 + s * 512:
                                       q * 1024 + (s + 1) * 512],
                                a0_sb[64 * s:64 * s + 64, j,
                                      m * 128:(m + 1) * 128],
                                hpT[64 * s:64 * s + 64, j, n_off:n_off + 512])

                def u_mms(m, q, et_sl, start, stop, j=j, u_ps=u_ps):
                    for s in range(2):
                        _mm(nc, u_ps[:, s * 1024 + q * 512:
                                     s * 1024 + (q + 1) * 512],
                            hp1[:, 2 * j + s, m, :],
                            et_sl[:, s * 512:(s + 1) * 512],
                            start=start, stop=stop)

                def finish(j=j, half=half, u_ps=u_ps):
                    # one batched division for both heads of the pair
                    r1 = small.tile([1, 2048], F32, tag="r1")
                    nc.vector.reciprocal(out=r1, in_=u_ps[64:65, :])
                    r1_dram = dram.tile([1, 2048], F32, tag="r1d")
                    nc.sync.dma_start(out=r1_dram, in_=r1)
                    rb_sb = ep.tile([64, 2048], F32, tag="e")
                    bc_ap = bass.AP(tensor=r1_dram.tensor,
                                    offset=r1_dram.offset,
                                    ap=[[0, 64], [1, 2048]])
                    nc.sync.dma_start(out=rb_sb, in_=bc_ap)
                    dst = x1T[:, 2 * j:2 * j + 2,
                              half * 1024:(half + 1) * 1024]
                    nc.vector.tensor_mul(
                        out=dst.bitcast(F32R),
                        in0=u_ps[0:64, :].rearrange("p (s n) -> p s n", s=2),
                        in1=rb_sb.rearrange("p (s n) -> p s n", s=2))

                attention(z_mms, u_mms, NT, finish)

        # deferred, batched gelu(x + bias0) in place (one ACT table switch)
        nc.scalar.activation(out=x1T.bitcast(F32R), in_=x1T,
                             func=gelu_func, bias=b0_sb)

        # ---------- instance norm 1 (per channel = (h, o)) ----------
        st1 = small.tile([64, 4, 6], F32, tag="st")
        mv1 = small.tile([64, H0, 2], F32, tag="mv1")
        sd1 = small.tile([64, H0], F32, tag="sd1")
        rs1 = small.tile([64, H0], F32, tag="rs1")
        for h in range(H0):
            for c in range(4):
                nc.vector.bn_stats(out=st1[:, c, :],
                                   in_=x1T[:, h, c * 512:(c + 1) * 512])
            nc.vector.bn_aggr(out=mv1[:, h, :], in_=st1)
        nc.scalar.activation(out=sd1, in_=mv1[:, :, 1],
                             func=mybir.ActivationFunctionType.Sqrt,
                             bias=eps_sb[0:64, :])
        nc.vector.reciprocal(out=rs1, in_=sd1)
        for h in range(H0):
            nc.vector.tensor_scalar(out=x1T[:, h, :].bitcast(F32R),
                                    in0=x1T[:, h, :],
                                    scalar1=mv1[:, h, 0:1],
                                    scalar2=rs1[:, h:h + 1],
                                    op0=AX.subtract, op1=AX.mult)

        # ---------- layer1 h': hpT1 duplicated on both partition halves ----
        hpT1 = sb.tile([128, N], F32)
        hp_ps1 = ps.tile([128, 2048], F32, tag="z", bufs=1)
        for c in range(4):
            for kh in range(4):
                _mm(nc, hp_ps1[:, c * 512:(c + 1) * 512],
                    w1_sb[:, kh, :], x1T[:, kh, c * 512:(c + 1) * 512],
                    start=(kh == 0), stop=(kh == 3))
        nc.scalar.copy(out=hpT1.bitcast(F32R), in_=hp_ps1)

        hp11 = sb.tile([128, NT, 65], F32)
        nc.vector.tensor_copy(out=hp11[:, :, 64].bitcast(F32R),
                              in_=ones_sb[:, 0:NT])
        for g in range(2):
            hpr_ps1 = ps.tile([128, 8, FO], F32, tag="z", bufs=1)
            for k in range(8):
                m = 8 * g + k
                nc.tensor.transpose(hpr_ps1[:, k, :],
                                    hpT1[0:64, m * 128:(m + 1) * 128],
                                    ident[0:64, 0:64])
            nc.vector.tensor_copy(
                out=hp11[:, 8 * g:8 * g + 8, 0:64].bitcast(F32R),
                in_=hpr_ps1)

        # ---------- layer1 attention (m-tile pairs) -> out ----------
        out_sb = sb.tile([128, NT, FO], F32)

        for half in range(2):
            u1_ps = ps.tile([65, 1024], F32, tag="u", bufs=1)

            def z_mms1(m2, zt, half=half):
                for q in range(2):
                    n_off = half * 1024 + q * 512
                    for s in range(2):
                        m = 2 * m2 + s
                        _mm(nc, zt[:, q * 1024 + s * 512:
                                   q * 1024 + (s + 1) * 512],
                            a1_sb[64 * s:64 * s + 64, m * 128:(m + 1) * 128],
                            hpT1[64 * s:64 * s + 64, n_off:n_off + 512])

            def u_mms1(m2, q, et_sl, start, stop, u1_ps=u1_ps):
                for s in range(2):
                    _mm(nc, u1_ps[:, q * 512:(q + 1) * 512],
                        hp11[:, 2 * m2 + s, :],
                        et_sl[:, s * 512:(s + 1) * 512],
                        start=(start and s == 0), stop=(stop and s == 1))

            def finish1(half=half, u1_ps=u1_ps):
                u1 = ep.tile([65, 2048], F32, tag="e")
                nc.scalar.copy(out=u1[0:64, 0:1024], in_=u1_ps[0:64, :])
                nc.vector.reciprocal(out=u1[64:65, 0:1024], in_=u1_ps[64:65, :])
                for g in range(2):
                    tr_ps = ps.tile([128, 4, 65], F32, tag="z", bufs=1)
                    for k in range(4):
                        t = 4 * g + k
                        nc.tensor.transpose(tr_ps[:, k, :],
                                            u1[:, t * 128:(t + 1) * 128],
                                            ident[0:65, 0:65])
                    for k in range(4):
                        t = half * 8 + 4 * g + k
                        nc.vector.tensor_scalar(
                            out=out_sb[:, t, :], in0=tr_ps[:, k, 0:64],
                            scalar1=tr_ps[:, k, 64:65], scalar2=None,
                            op0=AX.mult)

            attention(z_mms1, u_mms1, NT // 2, finish1)

        nc.sync.dma_start(out=out_d.ap().rearrange("(t p) f -> p t f", p=128),
                          in_=out_sb)


def _prep_host(inputs):
    """Host-side packing of weights into device layouts (replicated)."""
    f32 = np.float32
    asum0 = (np.asarray(inputs['a_src0'], f32)
             + np.asarray(inputs['a_dst0'], f32))        # [4, 64, n]
    a0 = np.empty((128, 2, N), f32)
    for h in range(H0):
        a0[64 * (h % 2):64 * (h % 2) + 64, h // 2, :] = asum0[h]
    w0r = np.asarray(inputs['w0'], f32)                  # [4, 64, 64]
    w0 = np.empty((64, 2, 128), f32)
    for j in range(2):
        w0[:, j, 0:64] = w0r[2 * j]
        w0[:, j, 64:128] = w0r[2 * j + 1]
    b0 = np.ascontiguousarray(np.asarray(inputs['bias0'], f32).reshape(64, 1))
    asum1 = (np.asarray(inputs['a_src1'], f32)
             + np.asarray(inputs['a_dst1'], f32))[0]     # [64, n]
    a1 = np.concatenate([asum1, asum1], axis=0)          # [128, n] dup
    w1r = np.asarray(inputs['w1'], f32)[0].reshape(4, 64, FO)
    w1 = np.empty((64, 4, 128), f32)
    for kh in range(4):
        w1[:, kh, 0:64] = w1r[kh]
        w1[:, kh, 64:128] = w1r[kh]
    return {'a0': np.ascontiguousarray(a0),
            'w0': np.ascontiguousarray(w0), 'b0': b0,
            'a1': np.ascontiguousarray(a1),
            'w1': np.ascontiguousarray(w1)}


_NC_CACHE = {}


def _get_nc(sim_safe=False):
    if sim_safe not in _NC_CACHE:
        _NC_CACHE[sim_safe] = build_bass(sim_safe=sim_safe)
    return _NC_CACHE[sim_safe]


LAST_RESULTS = None  # BassKernelResults of the last kernel() call


def kernel(**inputs):
    from concourse.bass_utils import run_bass_kernel_spmd
    global LAST_RESULTS

    nc = _get_nc(sim_safe=False)
    w = _prep_host(inputs)
    x = np.asarray(inputs['x'], dtype=np.float32)
    in_maps = [{'x': np.ascontiguousarray(x[i]), **w} for i in range(N_CORES)]

    res = run_bass_kernel_spmd(nc, in_maps, core_ids=list(range(N_CORES)))
    LAST_RESULTS = res
    out = np.stack([res.results[i]['out'] for i in range(N_CORES)])
    out = out + np.asarray(inputs['bias1'], dtype=np.float32)[None, None, :]
    return out.astype(np.float32)



# revision 4
# speedup vs baseline: 3.4990x; 3.4990x over previous
"""Trainium2 Bass kernel v2 for nn_GAT_79224966742097 — mega-loop design.

Backend empirical law: every top-level instruction costs ~50us to dispatch
regardless of size; instructions inside a For_i loop are only taxed for the
first ~210 dynamic executions (~11 ms total), then nearly free.  So the
whole two-layer GAT is ONE For_i loop of 25 iterations:

  i in [0,16): layer0 attention m-tile i   (accumulates U0 for real)
  every iter : h'0 projection (idempotent recompute)
  every iter : glue = U0/denom -> gelu -> instance-norm1 -> h'1
               (garbage before i=16, correct+idempotent from i=16)
  i in [17,25): layer1 attention m-pair (i+7)%8  (accumulates U1)

Accumulator writes are steered between the real accumulator and a scratch
sink via register arithmetic: off = scratch - sel*(scratch - real).

Precision: the layer0 -> x1 chain is all f32/f32r (layer1's exp amplifies
x1 errors ~3x, so bf16 anywhere upstream fails the 2e-2 gate); only
layer1's E/hpr operands are bf16.  Softmax denominators come from an
all-ones 65th lhsT column; 1/denom is broadcast across partitions via a
DRAM bounce (f32).  m-dependent stationary operands are DMA-staged into
fixed tiles each iteration (ldweights cannot take register offsets).
"""

import numpy as np

import concourse.bass as bass
import concourse.bacc as bacc
import concourse.mybir as mybir
import concourse.tile as tile
from contextlib import ExitStack
from concourse.masks import make_identity

F32 = mybir.dt.float32
F32R = mybir.dt.float32r
BF16 = mybir.dt.bfloat16
AX = mybir.AluOpType
AF = mybir.ActivationFunctionType
ds = bass.ds

N = 2048
EPS = 1e-5
NEG_SLOPE = 0.2
N_CORES = 8
N_ITER = 26
DEBUG = False


def build_bass(sim_safe=False, repeat=1):
    # sim_safe accepted for interface compatibility; unused.
    nc = bacc.Bacc("TRN2", debug=False)

    x_d = nc.dram_tensor("x", [N, 64], F32, kind="ExternalInput")
    a0_d = nc.dram_tensor("a0", [128, 2, N], F32, kind="ExternalInput")
    w0_d = nc.dram_tensor("w0", [64, 2, 128], F32, kind="ExternalInput")
    b0_d = nc.dram_tensor("b0", [64, 1], F32, kind="ExternalInput")
    a1_d = nc.dram_tensor("a1", [128, N], F32, kind="ExternalInput")
    w1_d = nc.dram_tensor("w1", [64, 4, 128], F32, kind="ExternalInput")
    out_d = nc.dram_tensor("out", [N, 64], F32, kind="ExternalOutput")
    dbg = {}
    if DEBUG:
        dbg['hpT'] = nc.dram_tensor("dbg_hpT", [128, 2, N], F32,
                                    kind="ExternalOutput")
        dbg['E0'] = nc.dram_tensor("dbg_E0", [128, 4, N], F32,
                                   kind="ExternalOutput")
        dbg['u0a'] = nc.dram_tensor("dbg_u0a", [65, 4 * N + 1024], F32,
                                    kind="ExternalOutput")
        dbg['x1T'] = nc.dram_tensor("dbg_x1T", [64, 4, N], F32,
                                    kind="ExternalOutput")
        dbg['hpT1'] = nc.dram_tensor("dbg_hpT1", [128, N], F32,
                                     kind="ExternalOutput")
        dbg['E1'] = nc.dram_tensor("dbg_E1", [128, 2, N], BF16,
                                   kind="ExternalOutput")
        dbg['u1a'] = nc.dram_tensor("dbg_u1a", [65, N + 1024], F32,
                                    kind="ExternalOutput")

    with tile.TileContext(nc) as tc, ExitStack() as ctx:
        const = ctx.enter_context(tc.tile_pool(name="const", bufs=1))
        sb = ctx.enter_context(tc.tile_pool(name="sb", bufs=1))
        ps = ctx.enter_context(tc.tile_pool(name="ps", bufs=1, space="PSUM"))
        dram = ctx.enter_context(tc.tile_pool(name="dram", bufs=1,
                                              space="DRAM"))
        for _rep in range(repeat):
            body(nc, tc, const, sb, ps, dram,
                 x_d, a0_d, w0_d, b0_d, a1_d, w1_d, out_d, dbg)
    nc.compile()
    return nc


def body(nc, tc, const, sb, ps, dram, x_d, a0_d, w0_d, b0_d, a1_d, w1_d,
         out_d, dbg):
    # ---------------- static prologue ----------------
    ident = const.tile([128, 128], F32, name="ident", uniquify=True)
    make_identity(nc, ident)

    eps_sb = const.tile([64, 1], F32, tag="eps")
    nc.vector.memset(eps_sb, EPS)

    b0_sb = const.tile([64, 1], F32, tag="b0")
    nc.sync.dma_start(out=b0_sb, in_=b0_d.ap())
    w0_sb = const.tile([64, 2, 128], F32, tag="w0")
    nc.sync.dma_start(out=w0_sb.bitcast(F32R), in_=w0_d.ap().bitcast(F32R))
    w1_sb = const.tile([64, 4, 128], F32, tag="w1")
    nc.sync.dma_start(out=w1_sb.bitcast(F32R), in_=w1_d.ap().bitcast(F32R))

    # x loaded transposed, instance-norm0 applied statically in place
    xT = sb.tile([64, N], F32, tag="xT")
    nc.sync.dma_start(out=xT.bitcast(F32R),
                      in_=x_d.ap().rearrange("n f -> f n").bitcast(F32R))
    st0 = sb.tile([64, 4, 6], F32, tag="st0")
    for c in range(4):
        nc.vector.bn_stats(out=st0[:, c, :], in_=xT[:, c * 512:(c + 1) * 512])
    mv0 = sb.tile([64, 2], F32, tag="mv0")
    nc.vector.bn_aggr(out=mv0, in_=st0)
    sd0 = sb.tile([64, 1], F32, tag="sd0")
    nc.scalar.activation(out=sd0, in_=mv0[:, 1:2], func=AF.Sqrt, bias=eps_sb)
    rs0 = sb.tile([64, 1], F32, tag="rs0")
    nc.vector.reciprocal(out=rs0, in_=sd0)
    nc.vector.tensor_scalar(out=xT.bitcast(F32R), in0=xT,
                            scalar1=mv0[:, 0:1], scalar2=rs0,
                            op0=AX.subtract, op1=AX.mult)

    hpT = sb.tile([128, 2, N], F32, tag="hpT")
    hpr = sb.tile([128, 2, 2, 65], F32, tag="hpr")
    ones4 = const.tile([128, 4], F32, tag="ones4")
    nc.vector.memset(ones4, 1.0)
    nc.vector.tensor_copy(out=hpr.bitcast(F32R)[:, :, :, 64], in_=ones4)
    E0 = sb.tile([128, 4, N], F32, tag="E0")
    u0a = sb.tile([65, 4 * N + 1024], F32, tag="u0a")
    nc.vector.memset(u0a, 0.0)

    d0d = dram.tile([1, 4 * N], F32, tag="d0d")
    D0b = sb.tile([64, 2 * N], F32, tag="D0b")
    x1T = sb.tile([64, 4, N], F32, tag="x1T")
    st1 = sb.tile([64, 4, 6], F32, tag="st1")
    mv1 = sb.tile([64, 4, 2], F32, tag="mv1")
    sd1 = sb.tile([64, 4], F32, tag="sd1")
    rs1 = sb.tile([64, 4], F32, tag="rs1")

    w1s = sb.tile([64, 4, 128], F32, tag="w1s")
    hpb = sb.tile([128, 1], F32, tag="hpb")
    hpT1 = sb.tile([128, N], F32, tag="hpT1")
    hpr1 = sb.tile([128, 2, 65], BF16, tag="hpr1")
    nc.vector.memset(hpr1[:, :, 64], 1.0)
    E1 = sb.tile([128, 2, N], BF16, tag="E1")
    u1a = sb.tile([65, N + 1024], F32, tag="u1a")
    nc.vector.memset(u1a, 0.0)

    # staging tiles for m-dependent stationary operands (ldweights cannot
    # take register offsets; DMA can)
    a0m = sb.tile([128, 2, 128], F32, tag="a0m")
    hpTm = sb.tile([128, 2, 128], F32, tag="hpTm")
    a1m = sb.tile([128, 256], F32, tag="a1m")
    hpT1m = sb.tile([128, 256], F32, tag="hpT1m")

    # ---- h'0 projection (static: hpT is loop-invariant) ----
    for j in range(2):
        for q in range(2):
            hp_ps = ps.tile([128, 1024], F32, tag="zf", bufs=2,
                            name="hp_ps")
            for c in range(2):
                off = q * 1024 + c * 512
                nc.tensor.matmul(
                    hp_ps[:, c * 512:(c + 1) * 512],
                    w0_sb[:, j, :].bitcast(F32R),
                    xT[:, off:off + 512].bitcast(F32R),
                    start=True, stop=True)
            dst = hpT[:, j, q * 1024:(q + 1) * 1024].bitcast(F32R)
            if q == 0:
                nc.scalar.copy(out=dst, in_=hp_ps)
            else:
                nc.vector.tensor_copy(out=dst, in_=hp_ps)

    hpT1_prev = sb.tile([128, N], F32, tag="hpT1p")

    # ---------------- the mega-loop ----------------
    with tc.For_i(0, N_ITER, 1) as i:
        m0c = (i % 16) * 128
        sel0 = 1 - (i // 16)
        mpc = ((i + 6) % 8) * 256
        sel1 = i // 18

        # ---- prefetch/stale reads first: all long-latency producers ----
        nc.sync.dma_start(out=a0m.bitcast(F32R),
                          in_=a0_d.ap()[:, :, ds(m0c, 128)].bitcast(F32R))
        nc.sync.dma_start(out=hpTm.bitcast(F32R),
                          in_=hpT[:, :, ds(m0c, 128)].bitcast(F32R))
        nc.sync.dma_start(out=a1m.bitcast(F32R),
                          in_=a1_d.ap()[:, ds(mpc, 256)].bitcast(F32R))
        # stale hpT1 (last iteration's glue): correct from i = 17
        nc.scalar.copy(out=hpT1_prev.bitcast(F32R), in_=hpT1)
        nc.sync.dma_start(out=hpT1m.bitcast(F32R),
                          in_=hpT1_prev[:, ds(mpc, 256)].bitcast(F32R))
        # stale denominator bounce (u0a row 64 is final from i = 16)
        nc.sync.dma_start(out=d0d, in_=u0a[64:65, 0:4 * N])
        # stale instance-norm1 stats of x1T (correct from i = 17)
        for h in range(4):
            for c in range(4):
                nc.vector.bn_stats(out=st1[:, c, :],
                                   in_=x1T[:, h, c * 512:(c + 1) * 512])
            nc.vector.bn_aggr(out=mv1[:, h, :], in_=st1)
        nc.scalar.activation(out=sd1, in_=mv1[:, :, 1], func=AF.Sqrt,
                             bias=eps_sb)
        nc.vector.reciprocal(out=rs1, in_=sd1)

        # ---- layer0 attention, m-tile i%16 ----
        tr_ps = ps.tile([128, 256], F32, tag="tr", bufs=1, name="tr_ps")
        for j in range(2):
            nc.tensor.transpose(tr_ps[:, j * 128:(j + 1) * 128],
                                hpTm[:, j, :], ident)
        nc.vector.tensor_copy(
            out=hpr.bitcast(F32R)[:, :, :, 0:64],
            in_=tr_ps.rearrange("p (j s o) -> p j s o", j=2, s=2))

        for j in range(2):
            for s in range(2):
                for q in range(2):
                    zt = ps.tile([128, 1024], F32, tag="zf", bufs=2,
                                 name="zt")
                    for c in range(2):
                        off = q * 1024 + c * 512
                        nc.tensor.matmul(
                            zt[:, c * 512:(c + 1) * 512],
                            a0m[64 * s:64 * s + 64, j, :].bitcast(F32R),
                            hpT[64 * s:64 * s + 64, j,
                                off:off + 512].bitcast(F32R),
                            start=True, stop=True)
                    nc.scalar.activation(
                        out=E0.bitcast(F32R)[:, 2 * j + s,
                                             q * 1024:(q + 1) * 1024],
                        in_=zt, func=AF.Prelu, alpha=NEG_SLOPE)
        nc.scalar.activation(out=E0.bitcast(F32R), in_=E0, func=AF.Exp)

        for j in range(2):
            for s in range(2):
                h = 2 * j + s
                for q in range(2):
                    u_ps = ps.tile([65, 1024], F32, tag="uf", bufs=1,
                                   name="u_ps")
                    for c in range(2):
                        off = q * 1024 + c * 512
                        nc.tensor.matmul(u_ps[:, c * 512:(c + 1) * 512],
                                         hpr[:, j, s, :].bitcast(F32R),
                                         E0[:, h,
                                            off:off + 512].bitcast(F32R),
                                         start=True, stop=True)
                    uoff = 4 * N - sel0 * (4 * N - (h * N + q * 1024))
                    nc.vector.tensor_tensor(out=u0a[:, ds(uoff, 1024)],
                                            in0=u0a[:, ds(uoff, 1024)],
                                            in1=u_ps, op=AX.add)

        # ---- glue (valid + idempotent from i = 16/17) ----
        for g in range(2):
            sl = slice(g * 2 * N, (g + 1) * 2 * N)
            bcg = bass.AP(tensor=d0d.tensor,
                          offset=d0d.offset + g * 2 * N,
                          ap=[[0, 64], [1, 2 * N]])
            nc.sync.dma_start(out=D0b, in_=bcg)
            nc.vector.reciprocal(out=D0b, in_=D0b)
            nc.vector.tensor_tensor(
                out=x1T.rearrange("p h n -> p (h n)").bitcast(F32R)[:, sl],
                in0=u0a[0:64, sl], in1=D0b, op=AX.mult)
        x1flat = x1T.rearrange("p h n -> p (h n)")
        nc.scalar.activation(out=x1flat.bitcast(F32R), in_=x1flat,
                             func=AF.Gelu, bias=b0_sb)

        # fold instance-norm1 into h'1: w1s = w1*rs (per channel), and
        # hpb[o'] = sum_c w1s[c,o']*mu_c subtracted at psum drain
        for kh in range(4):
            nc.vector.tensor_scalar(out=w1s.bitcast(F32R)[:, kh, :],
                                    in0=w1_sb[:, kh, :],
                                    scalar1=rs1[:, kh:kh + 1], scalar2=None,
                                    op0=AX.mult)
        hb_ps = ps.tile([128, 1], F32, tag="hb", bufs=1, name="hb_ps")
        for kh in range(4):
            nc.tensor.matmul(hb_ps, w1s[:, kh, :], mv1[:, kh, 0:1],
                             start=(kh == 0), stop=(kh == 3))
        nc.vector.tensor_copy(out=hpb, in_=hb_ps)

        for q in range(2):
            hp1_ps = ps.tile([128, 1024], F32, tag="zf", bufs=2,
                             name="hp1_ps")
            for c in range(2):
                for kh in range(4):
                    off = q * 1024 + c * 512
                    nc.tensor.matmul(
                        hp1_ps[:, c * 512:(c + 1) * 512],
                        w1s[:, kh, :].bitcast(F32R),
                        x1T[:, kh, off:off + 512].bitcast(F32R),
                        start=(kh == 0), stop=(kh == 3))
            dst1 = hpT1[:, q * 1024:(q + 1) * 1024].bitcast(F32R)
            nc.vector.tensor_scalar(out=dst1, in0=hp1_ps,
                                    scalar1=hpb, scalar2=None,
                                    op0=AX.subtract)

        # ---- layer1 attention, m-pair (i+6)%8 (stale h\'1) ----
        tr1_ps = ps.tile([128, 256], F32, tag="tr", bufs=1, name="tr1_ps")
        for s in range(2):
            nc.tensor.transpose(tr1_ps[:, s * 128:s * 128 + 64],
                                hpT1m[0:64, s * 128:(s + 1) * 128],
                                ident[0:64, 0:64])
        nc.vector.tensor_copy(
            out=hpr1[:, :, 0:64],
            in_=tr1_ps.rearrange("p (s o) -> p s o", s=2)[:, :, 0:64])

        for s in range(2):
            for q in range(2):
                zt1 = ps.tile([128, 1024], F32, tag="zf", bufs=2,
                              name="zt1")
                for c in range(2):
                    off = q * 1024 + c * 512
                    nc.tensor.matmul(
                        zt1[:, c * 512:(c + 1) * 512],
                        a1m[64 * s:64 * s + 64,
                            s * 128:(s + 1) * 128].bitcast(F32R),
                        hpT1_prev[64 * s:64 * s + 64,
                                  off:off + 512].bitcast(F32R),
                        start=True, stop=True)
                nc.scalar.activation(
                    out=E1[:, s, q * 1024:(q + 1) * 1024],
                    in_=zt1, func=AF.Prelu, alpha=NEG_SLOPE)
        nc.scalar.activation(out=E1, in_=E1, func=AF.Exp)

        for q in range(2):
            u1_ps = ps.tile([65, 1024], F32, tag="uf", bufs=1, name="u1_ps")
            for c in range(2):
                off = q * 1024 + c * 512
                for s in range(2):
                    nc.tensor.matmul(u1_ps[:, c * 512:(c + 1) * 512],
                                     hpr1[:, s, :],
                                     E1[:, s, off:off + 512],
                                     start=(s == 0), stop=(s == 1))
            u1off = N - sel1 * (N - q * 1024)
            nc.vector.tensor_tensor(out=u1a[:, ds(u1off, 1024)],
                                    in0=u1a[:, ds(u1off, 1024)],
                                    in1=u1_ps, op=AX.add)

    # ---------------- static epilogue ----------------
    d1d = dram.tile([1, N], F32, tag="d1d")
    nc.sync.dma_start(out=d1d, in_=u1a[64:65, 0:N])
    D1b = sb.tile([64, N], F32, tag="D1b")
    bc1 = bass.AP(tensor=d1d.tensor, offset=d1d.offset,
                  ap=[[0, 64], [1, N]])
    nc.sync.dma_start(out=D1b, in_=bc1)
    nc.vector.reciprocal(out=D1b, in_=D1b)
    outT = sb.tile([64, N], F32, tag="outT")
    nc.vector.tensor_tensor(out=outT, in0=u1a[0:64, 0:N], in1=D1b,
                            op=AX.mult)
    nc.sync.dma_start(out=out_d.ap().rearrange("n f -> f n"), in_=outT)
    if dbg:
        nc.sync.dma_start(out=dbg['hpT'].ap(), in_=hpT)
        nc.sync.dma_start(out=dbg['E0'].ap(), in_=E0)
        nc.sync.dma_start(out=dbg['u0a'].ap(), in_=u0a)
        nc.sync.dma_start(out=dbg['x1T'].ap(), in_=x1T)
        nc.sync.dma_start(out=dbg['hpT1'].ap(), in_=hpT1)
        nc.sync.dma_start(out=dbg['E1'].ap(), in_=E1)
        nc.sync.dma_start(out=dbg['u1a'].ap(), in_=u1a)


def _prep_host(inputs):
    f32 = np.float32
    asum0 = (np.asarray(inputs['a_src0'], f32)
             + np.asarray(inputs['a_dst0'], f32))        # [4, 64, n]
    a0 = np.empty((128, 2, N), f32)
    for h in range(4):
        a0[64 * (h % 2):64 * (h % 2) + 64, h // 2, :] = asum0[h]
    w0r = np.asarray(inputs['w0'], f32)                  # [4, 64, 64]
    w0 = np.empty((64, 2, 128), f32)
    for j in range(2):
        w0[:, j, 0:64] = w0r[2 * j]
        w0[:, j, 64:128] = w0r[2 * j + 1]
    b0 = np.ascontiguousarray(np.asarray(inputs['bias0'], f32).reshape(64, 1))
    asum1 = (np.asarray(inputs['a_src1'], f32)
             + np.asarray(inputs['a_dst1'], f32))[0]     # [64, n]
    a1 = np.concatenate([asum1, asum1], axis=0)          # [128, n]
    w1r = np.asarray(inputs['w1'], f32)[0].reshape(4, 64, 64)
    w1 = np.empty((64, 4, 128), f32)
    for kh in range(4):
        w1[:, kh, 0:64] = w1r[kh]
        w1[:, kh, 64:128] = w1r[kh]
    return {'a0': np.ascontiguousarray(a0),
            'w0': np.ascontiguousarray(w0), 'b0': b0,
            'a1': np.ascontiguousarray(a1),
            'w1': np.ascontiguousarray(w1)}


_NC_CACHE = {}


def _get_nc(sim_safe=False, repeat=1):
    # first positional arg kept for interface compatibility; unused.
    if repeat not in _NC_CACHE:
        _NC_CACHE[repeat] = build_bass(repeat=repeat)
    return _NC_CACHE[repeat]


def kernel(**inputs):
    from concourse.bass_utils import run_bass_kernel_spmd

    nc = _get_nc()
    w = _prep_host(inputs)
    x = np.asarray(inputs['x'], dtype=np.float32)
    in_maps = [{'x': np.ascontiguousarray(x[i]), **w} for i in range(N_CORES)]

    res = run_bass_kernel_spmd(nc, in_maps, core_ids=list(range(N_CORES)))
    out = np.stack([res.results[i]['out'] for i in range(N_CORES)])
    out = out + np.asarray(inputs['bias1'], dtype=np.float32)[None, None, :]
    return out.astype(np.float32)
